# revision 43
# baseline (speedup 1.0000x reference)
"""ROIPooling (adaptive 7x7 max pool over per-ROI valid h x w regions) on 8 trn2 cores.

Data-parallel over ROIs (64 per core, partition p = (roi, channel-half)).

Algorithm (v2), built around the TRN2 cost model:
  - bf16 end-to-end (tolerance is 2e-2; bf16 costs ~0.4%): halves DMA and
    enables the DVE high-performance modes.
  - Mirror trick: adaptive bins are mirror-symmetric (bin 6-b of x equals
    bin b of reversed-x), so the host sends rows [0..7] + reversed rows
    [0..5] (same 14 rows, gathered per-ROI on host).  Candidate slots per
    stage drop from 35 to 23 (chains of len 2,3,4,5 forward + 4,3,2
    mirrored); 21 of them need masks, 16 need max-links.
  - Layout [p, r, w, c] with channels innermost so every operand's last AP
    dim is packed: masked adds run as tensor_scalar (TSP, 4x mode,
    0.26ns/elem), max links as tensor_tensor (2x_1p, 0.52ns/elem) --
    vs 1.04ns/elem for the baseline's scalar_tensor_tensor chains.
  - A tunable subset of masked adds runs on the otherwise-idle ACT engine
    (activation Identity with per-partition fp32 bias mask) to balance
    engine busy time.  Masks stay fp32 (scalar operands are exempt from
    the 2x dtype rule).
  - Chunked input DMA with the first rows issued on three parallel queues
    (sync/scalar/gpsimd) to cut the cold-start ramp; per-column-bin output
    DMAs in chain-completion order hide the output tail.

Cost model: 39287 ns/core (baseline STT formulation: 106228 ns); HW
correctness rel err 3.1e-3 vs fp32 reference.
"""

import numpy as np
from contextlib import ExitStack

import ml_dtypes
import concourse.bass as bass
import concourse.bacc as bacc
import concourse.tile as tile
from concourse import mybir
from concourse.bass_utils import run_bass_kernel_spmd

N, C, H, W, OUT = 512, 256, 14, 14, 7
NCORES = 8
NS = N // NCORES          # 64 ROIs per core
CH = C // 2               # 128 channels innermost; 2 partitions per ROI
P = 128
NEG = -3.0e38

BF16 = mybir.dt.bfloat16
FP32 = mybir.dt.float32
BF16NP = np.dtype(ml_dtypes.bfloat16)

# chains: forward bins 0..3 read sent-rows b..b+k-1; mirrored bins (dest
# 6-b) read sent-rows 8+b..8+b+k-1.  k = b+2.  Slot (b, rr) is masked-in
# iff s_b(L) <= b+rr < e_b(L); the (b=0, rr=0) slot is always in.
CHAINS = [
    dict(b=0, base=0, k=2, dest=0),
    dict(b=1, base=1, k=3, dest=1),
    dict(b=2, base=2, k=4, dest=2),
    dict(b=3, base=3, k=5, dest=3),
    dict(b=2, base=10, k=4, dest=4),
    dict(b=1, base=9, k=3, dest=5),
    dict(b=0, base=8, k=2, dest=6),
]

# masked-slot enumeration order (shared host/device): chain-major, rr minor,
# skipping the always-in (b==0, rr==0) slots.
MASKED_SLOTS = []
for ci, ch_ in enumerate(CHAINS):
    for rr in range(ch_["k"]):
        if ch_["b"] == 0 and rr == 0:
            continue
        MASKED_SLOTS.append((ci, rr))
NMASK = len(MASKED_SLOTS)  # 21
MS_IDX = {sl: i for i, sl in enumerate(MASKED_SLOTS)}

# slots assigned to the ACT engine, per stage: (ci, rr) sets.
# Vertical slots are ~1792 elems (527ns DVE / 1678ns ACT), horizontal 896
# (294/932).  Balance: ~14 vertical + ~8 horizontal on ACT.
ACT_V = {(1, 0), (2, 0), (3, 0), (4, 0), (5, 0),
         (1, 2), (2, 2), (3, 2), (4, 2), (5, 2), (2, 3), (3, 3)}
ACT_H = {(1, 0), (2, 0), (3, 0), (4, 0), (5, 0),
         (1, 2), (2, 2), (3, 2), (4, 2), (5, 2), (4, 3)}


def _bins(L):
    i = np.arange(OUT)
    s = (i * L) // OUT
    e = ((i + 1) * L + OUT - 1) // OUT
    return s, e


def _mask_lut():
    """[8, NMASK] fp32 mask rows for L = 7..14."""
    lut = np.full((8, NMASK), NEG, np.float32)
    for li, L in enumerate(range(7, 15)):
        s, e = _bins(L)
        for i, (ci, rr) in enumerate(MASKED_SLOTS):
            b = CHAINS[ci]["b"]
            if s[b] <= b + rr < e[b]:
                lut[li, i] = 0.0
    return lut


_MLUT = _mask_lut()


CHUNKS = ((0, 2), (2, 4), (4, 6), (6, 8), (8, 11), (11, 14))


def _rowmap_lut():
    """[8, 14] gather indices: rows 0..7 then L-1 .. L-6."""
    lut = np.zeros((8, 14), np.int64)
    for li, L in enumerate(range(7, 15)):
        lut[li, :8] = np.arange(8)
        lut[li, 8:] = L - 1 - np.arange(6)
    return lut


_RLUT = _rowmap_lut()


def build_program():
    nc = bacc.Bacc("TRN2", target_bir_lowering=False, debug=False,
                   num_devices=NCORES)
    x = nc.dram_tensor("x", [P, H, W, CH], BF16, kind="ExternalInput").ap()
    vm = nc.dram_tensor("vm", [P, NMASK], FP32, kind="ExternalInput").ap()
    hm = nc.dram_tensor("hm", [P, NMASK], FP32, kind="ExternalInput").ap()
    out = nc.dram_tensor("out", [P, OUT, OUT, CH], BF16,
                         kind="ExternalOutput").ap()

    ADD = mybir.AluOpType.add
    MAX = mybir.AluOpType.max
    IDENT = mybir.ActivationFunctionType.Identity

    with tile.TileContext(nc) as tc, ExitStack() as ctx:
        pool = ctx.enter_context(tc.tile_pool(name="pool", bufs=1))

        # first x chunk before the (tiny) masks so compute starts ASAP;
        # finer chunks reduce wait-for-whole-chunk stalls.
        chunks = CHUNKS
        xtiles = {}
        xch = {}
        for r0, r1 in chunks:
            xtiles[r0] = pool.tile([P, r1 - r0, W, CH], BF16, name=f"x{r0}")
            for r in range(r0, r1):
                xch[r] = (xtiles[r0], r - r0)

        # first rows + masks issue on three idle queues in parallel to cut
        # the serial DGE/HWDGE latency off the critical path
        r0, r1 = chunks[0]
        nc.gpsimd.dma_start(xtiles[r0][:, r1 - r0 - 1:], x[:, r1 - 1:r1])
        nc.scalar.dma_start(xtiles[r0][:, : r1 - r0 - 1], x[:, r0:r1 - 1])
        vm_t = pool.tile([P, NMASK], FP32)
        nc.sync.dma_start(vm_t[:], vm)
        r0, r1 = chunks[1]
        nc.gpsimd.dma_start(xtiles[r0][:], x[:, r0:r1])
        hm_t = pool.tile([P, NMASK], FP32)
        nc.sync.dma_start(hm_t[:], hm)
        for r0, r1 in chunks[2:]:
            nc.sync.dma_start(xtiles[r0][:], x[:, r0:r1])

        def xrow(r):
            t, ofs = xch[r]
            return t[:, ofs]

        rowp = pool.tile([P, OUT, W, CH], BF16, name="rowp")
        out_t = pool.tile([P, OUT, OUT, CH], BF16, name="outt")

        # tmp tiles: DVE rotates a small pool (in-order engine resolves WAR);
        # ACT tmps are dedicated per slot so late folding never stalls ACT.
        vt_d = [pool.tile([P, W, CH], BF16, name=f"vtd{i}") for i in range(3)]
        ht_d = [pool.tile([P, OUT, CH], BF16, name=f"htd{i}") for i in range(3)]
        cnt = {"vd": 0, "hd": 0}

        # adjacent-dest ACT slots share one tmp tile; their folds merge into
        # a single TT over both chains' acc slices (dest(ci) == ci).
        PAIRS = []  # pair-merged folds measured slower: they lump the fold
        # pipeline and delay per-chain completion; DVE is gapless without them

        def emit_stage(src_of, acc_of, acc2_of, mask_t, act_set, tmps_d, kd,
                       shape, chains_sub_list, split_last=False, early=None):
            """One pooling stage, reassociated: DVE combines its own slots
            into acc first; ACT slot outputs (dedicated tmps) are folded in
            with late TTs so DVE never waits on an in-flight ACT op."""
            pairs = [p for p in PAIRS if p[0] in act_set and p[1] in act_set]
            pair_of = {}
            for a, b in pairs:
                pt = pool.tile([P, 2] + shape, BF16,
                               name=f"ap{kd}{a[0]}_{a[1]}")
                pair_of[a] = (pt, 0, a, b)
                pair_of[b] = (pt, 1, a, b)
            atmp = {}
            order = []
            # ACT ops first (by data availability) so the ACT queue runs ahead
            for chains_sub in chains_sub_list:
                for ci, rr in sorted(
                        ((ci, rr) for ci in chains_sub
                         for rr in range(CHAINS[ci]["k"])
                         if (ci, rr) in act_set),
                        key=lambda s: CHAINS[s[0]]["base"] + s[1]):
                    msk = mask_t[:, MS_IDX[(ci, rr)]: MS_IDX[(ci, rr)] + 1]
                    if (ci, rr) in pair_of:
                        pt, half, _, _ = pair_of[(ci, rr)]
                        dst = pt[:, half]
                    else:
                        t = pool.tile([P] + shape, BF16,
                                      name=f"at{kd}{ci}_{rr}")
                        atmp[(ci, rr)] = t
                        dst = t[:]
                    order.append((ci, rr))
                    nc.scalar.activation(out=dst, in_=src_of(ci, rr),
                                         func=IDENT, bias=msk, scale=1.0)
            # DVE phase 1: per-chain combine of DVE-owned slots
            started = {}
            early_done = early is None or early not in atmp
            early_folded = False
            for chains_sub in chains_sub_list:
                for ci, rr in sorted(
                        ((ci, rr) for ci in chains_sub
                         for rr in range(CHAINS[ci]["k"])
                         if (ci, rr) not in act_set),
                        key=lambda s: (CHAINS[s[0]]["base"] + s[1], s[0])):
                    if not early_done and CHAINS[ci]["base"] + rr > 1:
                        # fill the rows-2-3 DMA stall: fold the first ACT tmp
                        # (ready ~4.4us) into its chain's acc via self-max
                        eci = early[0]
                        te = atmp[early]
                        nc.vector.tensor_tensor(out=acc_of(eci), in0=te[:],
                                                in1=te[:], op=MAX)
                        started[eci] = True
                        early_done = True
                        early_folded = True
                    c = CHAINS[ci]
                    acc = acc_of(ci)
                    if c["b"] == 0 and rr == 0:
                        continue  # raw row; folded by the first TT below
                    msk = mask_t[:, MS_IDX[(ci, rr)]: MS_IDX[(ci, rr)] + 1]
                    if ci not in started:
                        if c["b"] == 0:
                            # masked slot + raw slot0 in one TSP+TT pair
                            tmp = tmps_d[cnt[kd] % len(tmps_d)]
                            cnt[kd] += 1
                            nc.vector.tensor_scalar(out=tmp[:], in0=src_of(ci, rr),
                                                    scalar1=msk, scalar2=None,
                                                    op0=ADD)
                            nc.vector.tensor_tensor(out=acc, in0=tmp[:],
                                                    in1=src_of(ci, 0), op=MAX)
                        else:
                            nc.vector.tensor_scalar(out=acc, in0=src_of(ci, rr),
                                                    scalar1=msk, scalar2=None,
                                                    op0=ADD)
                        started[ci] = True
                    else:
                        tmp = tmps_d[cnt[kd] % len(tmps_d)]
                        cnt[kd] += 1
                        nc.vector.tensor_scalar(out=tmp[:], in0=src_of(ci, rr),
                                                scalar1=msk, scalar2=None,
                                                op0=ADD)
                        nc.vector.tensor_tensor(out=acc, in0=tmp[:], in1=acc,
                                                op=MAX)
            # DVE phase 2: fold ACT tmps in exact ACT emission (= completion)
            # order so no fold waits while a later-emitted tmp sits ready.
            # Paired tmps fold once their later member is done.
            last_op = {}
            for i, (ci, rr) in enumerate(order):
                if early_folded and (ci, rr) == early:
                    continue  # folded early during the DMA stall
                if (ci, rr) in pair_of:
                    pt, _, a, b = pair_of[(ci, rr)]
                    other = b if (ci, rr) == a else a
                    if order.index(other) > i:
                        continue  # fold at the later member
                    acc2 = acc2_of(a[0])
                    nc.vector.tensor_tensor(out=acc2, in0=pt[:], in1=acc2,
                                            op=MAX)
                    last_op[a[0]] = len(last_op)
                    last_op[b[0]] = len(last_op)
                else:
                    acc = acc_of(ci)
                    t = atmp[(ci, rr)]
                    if split_last and i == len(order) - 1:
                        # final fold: two halves so the first half's output
                        # DMA overlaps the second half's compute
                        nc.vector.tensor_tensor(out=acc[:, 0:4], in0=t[:, 0:4],
                                                in1=acc[:, 0:4], op=MAX)
                        nc.vector.tensor_tensor(out=acc[:, 4:7], in0=t[:, 4:7],
                                                in1=acc[:, 4:7], op=MAX)
                        last_op[ci] = len(last_op)
                        last_op["split"] = ci
                        continue
                    nc.vector.tensor_tensor(out=acc, in0=t[:], in1=acc, op=MAX)
                    last_op[ci] = len(last_op)
            return last_op

        emit_stage(lambda ci, rr: xrow(CHAINS[ci]["base"] + rr)[:],
                   lambda ci: rowp[:, CHAINS[ci]["dest"]],
                   lambda ci: rowp[:, CHAINS[ci]["dest"]:CHAINS[ci]["dest"] + 2],
                   vm_t, ACT_V, vt_d, "vd", [W, CH], [[0, 1, 2, 3], [6, 5, 4]])

        h_last = emit_stage(lambda ci, rr: rowp[:, :, CHAINS[ci]["base"] + rr],
                            lambda ci: out_t[:, CHAINS[ci]["dest"]],
                            lambda ci: out_t[:, CHAINS[ci]["dest"]:
                                             CHAINS[ci]["dest"] + 2],
                            hm_t, ACT_H, ht_d, "hd", [OUT, CH],
                            [[0, 1, 2, 3], [6, 5, 4]], split_last=False)

        # output DMA per column chain, in completion order (fold-free chains
        # finish in phase 1); SP queue so the issue overhead doesn't occupy
        # the ACT engine.
        split_ci = h_last.pop("split", None)
        dma_order = [ci for ci in [0, 1, 2, 3, 6, 5, 4] if ci not in h_last]
        dma_order += sorted(h_last, key=h_last.get)
        for ci in dma_order:
            d = CHAINS[ci]["dest"]
            if ci == split_ci:
                nc.sync.dma_start(out[:, d, 0:4], out_t[:, d, 0:4])
                nc.sync.dma_start(out[:, d, 4:7], out_t[:, d, 4:7])
            else:
                nc.sync.dma_start(out[:, d], out_t[:, d])

        del xch, vt_d, ht_d

    nc.compile()
    return nc


def make_in_maps(rois, h, w):
    rois = np.ascontiguousarray(rois, np.float32).reshape(N, C, H, W)
    h = np.asarray(h).astype(np.int64)
    w = np.asarray(w).astype(np.int64)
    rmap = _RLUT[h - 7]            # [N, 14]
    cmap = _RLUT[w - 7]            # [N, 14]
    vmask = _MLUT[h - 7]           # [N, NMASK]
    hmask = _MLUT[w - 7]           # [N, NMASK]
    in_maps = []
    for k in range(NCORES):
        sl = slice(k * NS, (k + 1) * NS)
        xc = rois[sl]
        xg = np.take_along_axis(xc, rmap[sl][:, None, :, None], axis=2)
        xg = np.take_along_axis(xg, cmap[sl][:, None, None, :], axis=3)
        # [NS, C, H, W] -> [NS, 2, CH, H, W] -> [P, H, W, CH]
        xg = xg.reshape(NS, 2, CH, H, W).transpose(0, 1, 3, 4, 2) \
               .reshape(P, H, W, CH)
        in_maps.append({
            "x": np.ascontiguousarray(xg).astype(BF16NP),
            "vm": np.repeat(vmask[sl], 2, axis=0),
            "hm": np.repeat(hmask[sl], 2, axis=0),
        })
    return in_maps


_PROG = None


def kernel(rois, h, w):
    global _PROG
    if _PROG is None:
        _PROG = build_program()
    in_maps = make_in_maps(rois, h, w)
    res = run_bass_kernel_spmd(_PROG, in_maps, list(range(NCORES)))
    outs = []
    for k in range(NCORES):
        o = np.asarray(res.results[k]["out"]).astype(np.float32)
        # [P, out_col, out_row, CH] -> [NS*C, OUT, OUT]
        o = o.reshape(NS, 2, OUT, OUT, CH).transpose(0, 1, 4, 3, 2) \
             .reshape(NS * C, OUT, OUT)
        outs.append(o)
    return np.concatenate(outs, axis=0)
